# revision 49
# baseline (speedup 1.0000x reference)
"""Trainium2 Bass kernel for the PageRank-propagation problem.

out[i] = (1-C) * sum_j P[i,j] * t[j] + C/n
  P = |Bsym - sim|,  Bsym = triu(B,1) + triu(B,1).T
  t[j] = s[j] / rs[j],  rs[j] = sum_k P[j,k]

Sharding: rows split across 8 cores (1024 rows each).  TRANSPOSED tile
layout: each core stores P^T chunks [128 partitions = global column j,
1024 free = local row i], with global columns ROTATED by r0 so the SPMD
program is identical on every core: program chunk jc covers global
columns (r0 + jc*128 + p) mod n.  Chunks 0..7 are the diagonal band
(straddle resolved in the host packing by pure selection).

Phase 1 (DMA-bound, ~300-340 GB/s/core sustained): X and sim^T are
host-packed interleaved per chunk and streamed in 1 MiB groups on two
DMA queues (sync HWDGE + gpsimd SWDGE; NOT scalar, whose buffer-free
waits would stall the ACT drain stream).  Per chunk one of three
paths (PAT), balanced so every engine stays under the DMA roof:
  A: PE DoubleRow matmul (stationary = packed [+I | -I], moving =
     [X | sim] planes) -> PSUM f32, ACT Abs drain -> fp8
  V: same matmul, DVE copy drain + uint16 AND 0x7F7F abs
  D: DVE fp8 tensor_tensor subtract + uint16 AND abs (no PSUM)
rs accumulates in PSUM [1,1024] via DoubleRow ones-matmuls per chunk
pair, lagged one DMA group behind the drains so the in-order PE queue
never stalls.  All constants (identities, ones) are host inputs; no
gpsimd compute anywhere.

AllGather of the per-core row sums (2 KiB -> 16 KiB).  The band
chunks' GEMV needs only LOCAL rs, so its t-prep + 4 matmul pairs
overlap the collective (also keeps the PE HAM-warm).  Phase 2:
t = s * recip(rs) in a rotated [64,128] layout, prepared and
PE-transposed in two halves so the GEMV starts while half 2 is still
in flight; GEMV y = P^T.T @ t as DoubleRow matmuls with t chunk pairs
stationary; final affine on ACT.
"""

import sys

sys.path.insert(0, "/opt/trn_rl_repo")

import numpy as np

N = 8192
NCORES = 8
NB = N // NCORES          # rows per core (1024)
CW = 128                  # chunk width (columns per chunk = partitions)
NCH = N // CW             # 64 chunks per core
G = 8                     # chunks per DMA group
NG = NCH // G             # 8 groups
C = 0.15
TSCALE = 4096.0           # t is scaled into fp8 range; undone in final affine

_built = {}


def _build():
    if "nc" in _built:
        return _built["nc"]
    import concourse.bass as bass
    import concourse.bacc as bacc
    import concourse.tile as tile
    from concourse import mybir

    dt = mybir.dt
    Alu = mybir.AluOpType
    Act = mybir.ActivationFunctionType

    nc = bacc.Bacc(
        "TRN2", target_bir_lowering=False, debug=False, enable_asserts=False,
        num_devices=NCORES,
    )

    # X/sim interleaved per chunk: [group, partition, chunk, 2, NB]
    # plane 0 = X (Bsym^T content), plane 1 = sim^T
    XS = nc.dram_tensor("xs", [NG, 128, G, 2, NB], dt.float8e4,
                        kind="ExternalInput")
    # s rotated: srot[c, p] = s[(r0 + c*128 + p) % N]
    SROT = nc.dram_tensor("srot", [NCH, CW], dt.float32, kind="ExternalInput")
    # constants (host-built): [+I | -I] packed, id64, ones
    IDD = nc.dram_tensor("idd", [128, 2 * CW], dt.float8e4,
                         kind="ExternalInput")
    ID64 = nc.dram_tensor("id64", [64, 64], dt.bfloat16, kind="ExternalInput")
    ONES = nc.dram_tensor("ones", [128, 32], dt.float8e4,
                          kind="ExternalInput")
    OUT = nc.dram_tensor("out", [NB], dt.float32, kind="ExternalOutput")

    with tile.TileContext(nc, num_cores=NCORES) as tc:
        import contextlib

        with contextlib.ExitStack() as ctx:
            pp = ctx.enter_context(tc.tile_pool(name="pp", bufs=1))
            constp = ctx.enter_context(tc.tile_pool(name="constp", bufs=1))
            statp = ctx.enter_context(tc.tile_pool(name="statp", bufs=1))
            dramp = ctx.enter_context(
                tc.tile_pool(name="dramp", bufs=1, space="DRAM")
            )
            rspp = ctx.enter_context(
                tc.tile_pool(name="rspp", bufs=1, space="PSUM")
            )

            # ---- persistent tiles ----
            # P^T in fp8; two halves
            P_sba = pp.tile([128, (NCH // 2) * NB], dt.float8e4)
            P_sbb = pp.tile([128, (NCH // 2) * NB], dt.float8e4)

            idd = constp.tile([128, 2 * CW], dt.float8e4)
            id64 = constp.tile([64, 64], dt.bfloat16)
            # DoubleRow lhsT needs its two planes >=16B apart
            ones_f8 = constp.tile([128, 32], dt.float8e4)

            srot_sb = statp.tile([NCH, CW], dt.float32)
            rs_rot = statp.tile([NCH, CW], dt.bfloat16)
            trc = statp.tile([NCH, CW], dt.float32)
            t_stat = statp.tile([128, NCH], dt.float8e4)
            rs_sb = statp.tile([1, NB], dt.bfloat16)
            out_sb = statp.tile([1, NB], dt.float32)

            cc_in = dramp.tile([NB], dt.bfloat16)
            cc_out = dramp.tile([N], dt.bfloat16, addr_space="Shared")
            cc_dup = dramp.tile([2 * N], dt.bfloat16)
            # dummy-collective buffers: a tiny AllGather fired late in
            # phase 1 pre-warms the ncfw/TOPSP path so the real AllGather
            # skips most of its ~12us cold-wake latency
            cc_wi = dramp.tile([16], dt.bfloat16)
            cc_wo = dramp.tile([16 * NCORES], dt.bfloat16,
                               addr_space="Shared")

            # rs accumulator: [1, 1024] f32 = 2 PSUM banks, pinned all of
            # phase 1
            rs_ps = rspp.tile([1, NB], dt.float32)

            def P_chunk(jc):
                """P slice for chunk jc (full 1024 free cols)."""
                half = P_sba if jc < NCH // 2 else P_sbb
                base = (jc % (NCH // 2)) * NB
                return half[:, base: base + NB]

            def P_pair(jp, h):
                """DoubleRow moving AP: chunks (2jp, 2jp+1), cols [h:h+512)."""
                half = P_sba if 2 * jp < NCH // 2 else P_sbb
                base = ((2 * jp) % (NCH // 2)) * NB
                v = half[:, base: base + 2 * NB].rearrange(
                    "p (two n) -> p two n", two=2
                )
                return v[:, :, h:h + 512]

            # ---- constants off the bulk queues: idd (gates the first
            # matmul) on the still-empty scalar queue, the rest on gpsimd
            nc.scalar.dma_start(out=idd[:], in_=IDD.ap())
            nc.gpsimd.dma_start(out=id64[:], in_=ID64.ap())
            nc.gpsimd.dma_start(out=ones_f8[:], in_=ONES.ap())
            nc.gpsimd.dma_start(out=srot_sb[:], in_=SROT.ap())

            # stationary APs
            idd_dr = idd[:].rearrange("p (a m) -> p a m", a=2)
            ones_dr = ones_f8[:].rearrange("p (a m) -> p a m", a=2)[:, :, 0:1]

            rs_n = [0]

            def rs_pair(jp):
                """Emit DoubleRow row-sum MMs for completed chunk pair jp."""
                rs_n[0] += 1
                first = rs_n[0] == 1
                last = rs_n[0] == NCH // 2
                for h in (0, 512):
                    nc.tensor.matmul(
                        rs_ps[:, h:h + 512], ones_dr, P_pair(jp, h),
                        start=first, stop=last,
                        perf_mode=mybir.MatmulPerfMode.DoubleRow,
                    )

            # ---- phase 1 ----
            # per-chunk path (repeating length-8 pattern):
            #   A = PE DR-subtract -> bf16 PSUM -> ACT Abs drain to fp8
            #   V = PE DR-subtract -> bf16 PSUM -> DVE copy + uint16 AND abs
            #   D = DVE tensor_tensor subtract (fp8) + uint16 AND abs
            PAT = "AAVADAADAADAVADD"
            with contextlib.ExitStack() as p1:
                stgp = p1.enter_context(tc.tile_pool(name="stgp", bufs=6))
                psp = p1.enter_context(
                    tc.tile_pool(name="psp", bufs=6, space="PSUM")
                )

                for g in range(NG):
                    st = stgp.tile([128, G, 2, NB], dt.float8e4, tag="st")
                    # ALL bulk DMAs on the sync HWDGE queue: scalar would
                    # stall the ACT drains behind a buffer-free wait, and
                    # gpsimd must stay free to fire the warm-up collective
                    # early (its completion wait parks that queue)
                    if g == 0:
                        # split finely so the first chunks' matmuls start
                        # as soon as possible
                        for q in range(4):
                            nc.sync.dma_start(
                                out=st[:, q * G // 4:(q + 1) * G // 4],
                                in_=XS[0, :, q * G // 4:(q + 1) * G // 4],
                            )
                    elif g in (1, NG - 1):
                        # halve g1 (ramp: its first chunks otherwise wait
                        # behind 3 MiB) and the last group (shortens the
                        # phase-1 drain-out before the rs handoff)
                        for q in range(2):
                            nc.sync.dma_start(
                                out=st[:, q * G // 2:(q + 1) * G // 2],
                                in_=XS[g, :, q * G // 2:(q + 1) * G // 2],
                            )
                    else:
                        nc.sync.dma_start(out=st[:], in_=XS[g])
                    if g == 1:
                        # warm-up collective: pre-wakes ncfw (cuts the
                        # real AllGather's ~12us cold-wake to ~1us) and
                        # absorbs cross-core start skew inside phase 1.
                        # Input sourced from this stage tile so Tile
                        # cannot hoist the trigger before phase 1.
                        nc.sync.dma_start(
                            out=cc_wi[:].rearrange("(a f) -> a f", a=1),
                            in_=st[0:1, 0, 0, 0:32].bitcast(dt.bfloat16),
                        )
                        nc.gpsimd.collective_compute(
                            "AllGather", Alu.bypass,
                            replica_groups=[list(range(NCORES))],
                            ins=[cc_wi[:]], outs=[cc_wo[:]],
                        )
                    if g > 0:
                        # rs for the previous group's pairs: lagged one
                        # group so the PE queue never stalls on drains
                        for jp in range((g - 1) * G // 2, g * G // 2):
                            rs_pair(jp)
                    for a in range(G):
                        jc = g * G + a
                        pc = P_chunk(jc)
                        path = PAT[jc % len(PAT)]
                        if path == "D":
                            nc.vector.tensor_tensor(
                                out=pc, in0=st[:, a, 0, :], in1=st[:, a, 1, :],
                                op=Alu.subtract,
                            )
                            pcu = pc.bitcast(dt.uint16)
                            nc.vector.tensor_scalar(
                                out=pcu, in0=pcu, scalar1=0x7F7F,
                                scalar2=None, op0=Alu.bitwise_and,
                            )
                        else:
                            # one PSUM bank per half-chunk: finer drain
                            # granularity (drain h0 while h1's MM runs)
                            # and two fd=512 drains beat one fd=1024
                            for h in (0, 512):
                                ps = psp.tile(
                                    [128, 512], dt.float32, tag="ps"
                                )
                                nc.tensor.matmul(
                                    ps[:], idd_dr,
                                    st[:, a, :, h:h + 512],
                                    start=True, stop=True,
                                    perf_mode=mybir.MatmulPerfMode.DoubleRow,
                                )
                                if path == "A":
                                    nc.scalar.activation(
                                        out=pc[:, h:h + 512], in_=ps[:],
                                        func=Act.Abs,
                                    )
                                else:
                                    nc.vector.tensor_copy(
                                        out=pc[:, h:h + 512], in_=ps[:]
                                    )
                            if path == "V":
                                pcu = pc.bitcast(dt.uint16)
                                nc.vector.tensor_scalar(
                                    out=pcu, in0=pcu, scalar1=0x7F7F,
                                    scalar2=None, op0=Alu.bitwise_and,
                                )
                # final group's rs pairs
                for jp in range((NG - 1) * G // 2, NG * G // 2):
                    rs_pair(jp)

            # ---- row sums -> AllGather (natural order); the PSUM->SBUF
            # copy is single-lane, so split it across ACT+DVE halves to
            # halve the latency on the collective-trigger path ----
            nc.scalar.activation(
                out=rs_sb[:, 0:512], in_=rs_ps[:, 0:512], func=Act.Copy
            )
            nc.vector.tensor_copy(
                out=rs_sb[:, 512:NB], in_=rs_ps[:, 512:NB]
            )
            nc.sync.dma_start(
                out=cc_in[:].rearrange("(a f) -> a f", a=1), in_=rs_sb[:]
            )
            nc.gpsimd.collective_compute(
                "AllGather", Alu.bypass,
                replica_groups=[list(range(NCORES))],
                ins=[cc_in[:]], outs=[cc_out[:]],
            )

            r0v = nc.partition_id() * NB
            NP2 = NCH // 2  # 32 chunk pairs
            BP = (NB // CW) // 2  # 4 band pairs

            with contextlib.ExitStack() as p2:
                psp2 = p2.enter_context(
                    tc.tile_pool(name="psp2", bufs=1, space="PSUM")
                )
                tp_ps = psp2.tile([128, NCH], dt.bfloat16)
                tp8_ps = psp2.tile([128, 8], dt.bfloat16)
                y_ps = psp2.tile([1, NB], dt.float32)

                t_split = t_stat[:].rearrange("p (a jp) -> p a jp", a=2)

                def gemv(jp0, jp1):
                    for jp in range(jp0, jp1):
                        t_dr = t_split[:, :, jp:jp + 1]
                        for h in (0, 512):
                            nc.tensor.matmul(
                                y_ps[:, h:h + 512], t_dr, P_pair(jp, h),
                                start=(jp == 0), stop=(jp == NP2 - 1),
                                perf_mode=mybir.MatmulPerfMode.DoubleRow,
                            )

                # ---- band chunks (own columns): t needs only LOCAL rs,
                # so this entire sub-GEMV overlaps the AllGather ----
                rs_b8 = statp.tile([8, CW], dt.bfloat16)
                trc8 = statp.tile([8, CW], dt.float32)
                t8_bf = statp.tile([8, CW], dt.bfloat16)
                nc.scalar.dma_start(
                    out=rs_b8[:],
                    in_=cc_in[0:NB].rearrange("(c p) -> c p", c=8),
                )
                nc.vector.reciprocal(out=trc8[:], in_=rs_b8[:])
                nc.vector.tensor_tensor(
                    out=trc8[:], in0=trc8[:], in1=srot_sb[0:8, :], op=Alu.mult
                )
                nc.scalar.activation(
                    out=t8_bf[:], in_=trc8[:], func=Act.Copy, scale=TSCALE,
                )
                nc.tensor.transpose(tp8_ps[:], t8_bf[:], id64[0:8, 0:8])
                nc.vector.tensor_copy(
                    out=t_split[:, :, 0:BP],
                    in_=tp8_ps[:].rearrange("p (jp a) -> p a jp", a=2),
                )
                gemv(0, BP)

                # ---- t = s * recip(rs), rotated, to stationary layout ----
                nc.sync.dma_start(out=cc_dup[0:N], in_=cc_out[:])
                nc.scalar.dma_start(
                    out=cc_dup[N:N + 7 * NB], in_=cc_out[0:7 * NB]
                )
                # two half-window loads on two queues: recip of half 1
                # starts as soon as its 8 KB lands
                nc.sync.dma_start(
                    out=rs_rot[0:32, :],
                    in_=cc_dup[bass.ds(r0v, N // 2)].rearrange(
                        "(c p) -> c p", c=NCH // 2
                    ),
                )
                nc.scalar.dma_start(
                    out=rs_rot[32:64, :],
                    in_=cc_dup[bass.ds(r0v + N // 2, N // 2)].rearrange(
                        "(c p) -> c p", c=NCH // 2
                    ),
                )
                # two halves so the GEMV starts while the second half's
                # t is still being prepared
                t_hs = [
                    statp.tile([32, CW], dt.bfloat16, name=f"t_h{i}")
                    for i in range(2)
                ]
                for hi, (c0, c1) in enumerate(((0, 32), (32, 64))):
                    # j0: first non-band chunk of this half
                    j0 = max(c0, 8)
                    nc.vector.reciprocal(
                        out=trc[c0:c1, :], in_=rs_rot[c0:c1, :]
                    )
                    nc.vector.tensor_tensor(
                        out=trc[c0:c1, :], in0=trc[c0:c1, :],
                        in1=srot_sb[c0:c1, :], op=Alu.mult,
                    )
                    # scale t into fp8 range (undone in the final affine)
                    nc.scalar.activation(
                        out=t_hs[hi][:], in_=trc[c0:c1, :],
                        func=Act.Copy, scale=TSCALE,
                    )
                    # transpose [32, 128] -> [128, 32]
                    nc.tensor.transpose(
                        tp_ps[:, c0:c1], t_hs[hi][:], id64[0:32, 0:32],
                    )
                    # parity-split copy: t_stat[p, a*32+jp] = t[chunk 2jp+a]
                    # (DoubleRow lhsT planes must be >=16B apart); skip the
                    # band cols already placed above
                    nc.vector.tensor_copy(
                        out=t_split[:, :, j0 // 2:c1 // 2],
                        in_=tp_ps[:, c0:c1].rearrange(
                            "p (jp a) -> p a jp", a=2
                        )[:, :, (j0 - c0) // 2:],
                    )
                    # ---- phase 2: GEMV y[i] = sum_j P^T[j,i] t[j] ----
                    gemv(j0 // 2, c1 // 2)

                # out = (1-C)/TSCALE * y' + C/n, split ACT+DVE (single-
                # lane op; halves the final-affine latency)
                nc.scalar.activation(
                    out=out_sb[:, 0:512], in_=y_ps[:, 0:512], func=Act.Copy,
                    bias=float(C / N), scale=float((1.0 - C) / TSCALE),
                )
                nc.vector.tensor_scalar(
                    out=out_sb[:, 512:NB], in0=y_ps[:, 512:NB],
                    scalar1=float((1.0 - C) / TSCALE),
                    scalar2=float(C / N),
                    op0=Alu.mult, op1=Alu.add,
                )
            nc.sync.dma_start(
                out=OUT.ap().rearrange("(a f) -> a f", a=1), in_=out_sb[:]
            )

    nc.finalize()
    _built["nc"] = nc
    return nc


def _shard_inputs(B, sim, s):
    """Host packing: slice / transpose / select / pack in fp8, plus the
    constant tables the kernel loads (identities, ones)."""
    import ml_dtypes

    f8 = ml_dtypes.float8_e4m3
    bf16 = ml_dtypes.bfloat16
    B8 = B.astype(f8).view(np.uint8)
    sim8 = sim.astype(f8).view(np.uint8)
    s_ext = np.concatenate([s, s]).astype(np.float32)

    idd = np.zeros((128, 2 * CW), dtype=np.float32)
    idd[:, 0:CW] = np.eye(CW)
    idd[:, CW:2 * CW] = -np.eye(CW)
    idd = idd.astype(f8)
    id64 = np.eye(64, dtype=np.float32).astype(bf16)
    ones = np.ones((128, 32), dtype=np.float32).astype(f8)

    iidx = np.arange(NB)[None, :]

    in_maps = []
    for d in range(NCORES):
        r0, r1 = d * NB, (d + 1) * NB

        xs = np.empty((NG, 128, G, 2, NB), dtype=np.uint8)
        Bblk = B8[r0:r1, r0:r1]
        for jc in range(NCH):
            j0 = (r0 + jc * CW) % N
            if jc < NB // CW:
                # band chunk: straddle resolved by selection
                cs = slice(jc * CW, (jc + 1) * CW)
                m1t = np.ascontiguousarray(Bblk[:, cs].T)
                m2 = Bblk[cs, :]
                pidx = (jc * CW + np.arange(CW))[:, None]
                x = np.where(iidx < pidx, m1t, np.where(iidx > pidx, m2, 0))
            elif j0 >= r1:
                x = np.ascontiguousarray(B8[r0:r1, j0:j0 + CW].T)
            else:
                x = B8[j0:j0 + CW, r0:r1]
            st = np.ascontiguousarray(sim8[r0:r1, j0:j0 + CW].T)
            g, a = jc // G, jc % G
            xs[g, :, a, 0, :] = x
            xs[g, :, a, 1, :] = st

        srot = np.ascontiguousarray(s_ext[r0:r0 + N].reshape(NCH, CW))

        in_maps.append({
            "xs": xs.view(f8),
            "srot": srot,
            "idd": idd,
            "id64": id64,
            "ones": ones,
        })
    return in_maps


def kernel(B, similarity_matrix, connectivity_scores, _trace=False,
           _tmpdir=None):
    from concourse import bass_utils

    B = np.asarray(B, dtype=np.float32)
    sim = np.asarray(similarity_matrix, dtype=np.float32)
    s = np.asarray(connectivity_scores, dtype=np.float32)

    nc = _build()
    in_maps = _shard_inputs(B, sim, s)
    res = bass_utils.run_bass_kernel_spmd(
        nc, in_maps, core_ids=list(range(NCORES)), trace=_trace,
        tmpdir=_tmpdir
    )
    out = np.concatenate([res.results[d]["out"] for d in range(NCORES)])
    if _trace:
        kernel.last_results = res
    return out


# revision 50
# speedup vs baseline: 1.0451x; 1.0451x over previous
"""Trainium2 Bass kernel for the PageRank-propagation problem.

out[i] = (1-C) * sum_j P[i,j] * t[j] + C/n
  P = |Bsym - sim|,  Bsym = triu(B,1) + triu(B,1).T
  t[j] = s[j] / rs[j],  rs[j] = sum_k P[j,k]

Sharding: rows split across 8 cores (1024 rows each).  TRANSPOSED tile
layout: each core stores P^T chunks [128 partitions = global column j,
1024 free = local row i], with global columns ROTATED by r0 so the SPMD
program is identical on every core: program chunk jc covers global
columns (r0 + jc*128 + p) mod n.  Chunks 0..7 are the diagonal band
(straddle resolved in the host packing by pure selection).

Phase 1 (DMA-bound, ~300-340 GB/s/core sustained): X and sim^T are
host-packed interleaved per chunk and streamed in 1 MiB groups on two
DMA queues (sync HWDGE + gpsimd SWDGE; NOT scalar, whose buffer-free
waits would stall the ACT drain stream).  Per chunk one of three
paths (PAT), balanced so every engine stays under the DMA roof:
  A: PE DoubleRow matmul (stationary = packed [+I | -I], moving =
     [X | sim] planes) -> PSUM f32, ACT Abs drain -> fp8
  V: same matmul, DVE copy drain + uint16 AND 0x7F7F abs
  D: DVE fp8 tensor_tensor subtract + uint16 AND abs (no PSUM)
rs accumulates in PSUM [1,1024] via DoubleRow ones-matmuls per chunk
pair, lagged one DMA group behind the drains so the in-order PE queue
never stalls.  All constants (identities, ones) are host inputs; no
gpsimd compute anywhere.

AllGather of the per-core row sums (2 KiB -> 16 KiB).  The band
chunks' GEMV needs only LOCAL rs, so its t-prep + 4 matmul pairs
overlap the collective (also keeps the PE HAM-warm).  Phase 2:
t = s * recip(rs) in a rotated [64,128] layout, prepared and
PE-transposed in two halves so the GEMV starts while half 2 is still
in flight; GEMV y = P^T.T @ t as DoubleRow matmuls with t chunk pairs
stationary; final affine on ACT.
"""

import sys

sys.path.insert(0, "/opt/trn_rl_repo")

import numpy as np

N = 8192
NCORES = 8
NB = N // NCORES          # rows per core (1024)
CW = 128                  # chunk width (columns per chunk = partitions)
NCH = N // CW             # 64 chunks per core
G = 8                     # chunks per DMA group
NG = NCH // G             # 8 groups
C = 0.15
TSCALE = 4096.0           # t is scaled into fp8 range; undone in final affine

_built = {}


def _build():
    if "nc" in _built:
        return _built["nc"]
    import concourse.bass as bass
    import concourse.bacc as bacc
    import concourse.tile as tile
    from concourse import mybir

    dt = mybir.dt
    Alu = mybir.AluOpType
    Act = mybir.ActivationFunctionType

    nc = bacc.Bacc(
        "TRN2", target_bir_lowering=False, debug=False, enable_asserts=False,
        num_devices=NCORES,
    )

    # X/sim interleaved per chunk: [group, partition, chunk, 2, NB]
    # plane 0 = X (Bsym^T content), plane 1 = sim^T
    XS = nc.dram_tensor("xs", [NG, 128, G, 2, NB], dt.float8e4,
                        kind="ExternalInput")
    # s rotated: srot[c, p] = s[(r0 + c*128 + p) % N]
    SROT = nc.dram_tensor("srot", [NCH, CW], dt.float32, kind="ExternalInput")
    # constants (host-built): [+I | -I] packed, id64, ones
    IDD = nc.dram_tensor("idd", [128, 2 * CW], dt.float8e4,
                         kind="ExternalInput")
    ID64 = nc.dram_tensor("id64", [64, 64], dt.bfloat16, kind="ExternalInput")
    ONES = nc.dram_tensor("ones", [128, 32], dt.float8e4,
                          kind="ExternalInput")
    OUT = nc.dram_tensor("out", [NB], dt.float32, kind="ExternalOutput")

    with tile.TileContext(nc, num_cores=NCORES) as tc:
        import contextlib

        with contextlib.ExitStack() as ctx:
            pp = ctx.enter_context(tc.tile_pool(name="pp", bufs=1))
            constp = ctx.enter_context(tc.tile_pool(name="constp", bufs=1))
            statp = ctx.enter_context(tc.tile_pool(name="statp", bufs=1))
            dramp = ctx.enter_context(
                tc.tile_pool(name="dramp", bufs=1, space="DRAM")
            )
            rspp = ctx.enter_context(
                tc.tile_pool(name="rspp", bufs=1, space="PSUM")
            )

            # ---- persistent tiles ----
            # P^T in fp8; two halves
            P_sba = pp.tile([128, (NCH // 2) * NB], dt.float8e4)
            P_sbb = pp.tile([128, (NCH // 2) * NB], dt.float8e4)

            idd = constp.tile([128, 2 * CW], dt.float8e4)
            id64 = constp.tile([64, 64], dt.bfloat16)
            # DoubleRow lhsT needs its two planes >=16B apart
            ones_f8 = constp.tile([128, 32], dt.float8e4)

            srot_sb = statp.tile([NCH, CW], dt.float32)
            rs_rot = statp.tile([NCH, CW], dt.bfloat16)
            trc = statp.tile([NCH, CW], dt.float32)
            t_stat = statp.tile([128, NCH], dt.float8e4)
            rs_sb = statp.tile([1, NB], dt.bfloat16)
            out_sb = statp.tile([1, NB], dt.float32)

            cc_in = dramp.tile([NB], dt.bfloat16)
            cc_out = dramp.tile([N], dt.bfloat16, addr_space="Shared")
            cc_dup = dramp.tile([2 * N], dt.bfloat16)
            # dummy-collective buffers: a tiny AllGather fired late in
            # phase 1 pre-warms the ncfw/TOPSP path so the real AllGather
            # skips most of its ~12us cold-wake latency
            cc_wi = dramp.tile([16], dt.bfloat16)
            cc_wo = dramp.tile([16 * NCORES], dt.bfloat16,
                               addr_space="Shared")

            # rs accumulator: [1, 1024] f32 = 2 PSUM banks, pinned all of
            # phase 1
            rs_ps = rspp.tile([1, NB], dt.float32)

            def P_chunk(jc):
                """P slice for chunk jc (full 1024 free cols)."""
                half = P_sba if jc < NCH // 2 else P_sbb
                base = (jc % (NCH // 2)) * NB
                return half[:, base: base + NB]

            def P_pair(jp, h):
                """DoubleRow moving AP: chunks (2jp, 2jp+1), cols [h:h+512)."""
                half = P_sba if 2 * jp < NCH // 2 else P_sbb
                base = ((2 * jp) % (NCH // 2)) * NB
                v = half[:, base: base + 2 * NB].rearrange(
                    "p (two n) -> p two n", two=2
                )
                return v[:, :, h:h + 512]

            # ---- constants off the bulk queues: idd (gates the first
            # matmul) on the still-empty scalar queue, the rest on gpsimd
            nc.scalar.dma_start(out=idd[:], in_=IDD.ap())
            nc.gpsimd.dma_start(out=id64[:], in_=ID64.ap())
            nc.gpsimd.dma_start(out=ones_f8[:], in_=ONES.ap())
            nc.gpsimd.dma_start(out=srot_sb[:], in_=SROT.ap())

            # stationary APs
            idd_dr = idd[:].rearrange("p (a m) -> p a m", a=2)
            ones_dr = ones_f8[:].rearrange("p (a m) -> p a m", a=2)[:, :, 0:1]

            rs_n = [0]

            def rs_pair(jp):
                """Emit DoubleRow row-sum MMs for completed chunk pair jp."""
                rs_n[0] += 1
                first = rs_n[0] == 1
                last = rs_n[0] == NCH // 2
                for h in (0, 512):
                    nc.tensor.matmul(
                        rs_ps[:, h:h + 512], ones_dr, P_pair(jp, h),
                        start=first, stop=last,
                        perf_mode=mybir.MatmulPerfMode.DoubleRow,
                    )

            # ---- phase 1 ----
            # per-chunk path (repeating length-8 pattern):
            #   A = PE DR-subtract -> bf16 PSUM -> ACT Abs drain to fp8
            #   V = PE DR-subtract -> bf16 PSUM -> DVE copy + uint16 AND abs
            #   D = DVE tensor_tensor subtract (fp8) + uint16 AND abs
            PAT = "AAVADAADAADAVADD"
            with contextlib.ExitStack() as p1:
                stgp = p1.enter_context(tc.tile_pool(name="stgp", bufs=7))
                psp = p1.enter_context(
                    tc.tile_pool(name="psp", bufs=6, space="PSUM")
                )

                for g in range(NG):
                    st = stgp.tile([128, G, 2, NB], dt.float8e4, tag="st")
                    # ALL bulk DMAs on the sync HWDGE queue: scalar would
                    # stall the ACT drains behind a buffer-free wait, and
                    # gpsimd must stay free to fire the warm-up collective
                    # early (its completion wait parks that queue)
                    if g == 0:
                        # split finely so the first chunks' matmuls start
                        # as soon as possible
                        for q in range(4):
                            nc.sync.dma_start(
                                out=st[:, q * G // 4:(q + 1) * G // 4],
                                in_=XS[0, :, q * G // 4:(q + 1) * G // 4],
                            )
                    elif g in (1, NG - 1):
                        # halve g1 (ramp: its first chunks otherwise wait
                        # behind 3 MiB) and the last group (shortens the
                        # phase-1 drain-out before the rs handoff)
                        for q in range(2):
                            nc.sync.dma_start(
                                out=st[:, q * G // 2:(q + 1) * G // 2],
                                in_=XS[g, :, q * G // 2:(q + 1) * G // 2],
                            )
                    else:
                        nc.sync.dma_start(out=st[:], in_=XS[g])
                    if g == 1:
                        # warm-up collective: pre-wakes ncfw (cuts the
                        # real AllGather's ~12us cold-wake to ~1us) and
                        # absorbs cross-core start skew inside phase 1.
                        # Input sourced from this stage tile so Tile
                        # cannot hoist the trigger before phase 1.
                        nc.sync.dma_start(
                            out=cc_wi[:].rearrange("(a f) -> a f", a=1),
                            in_=st[0:1, 0, 0, 0:32].bitcast(dt.bfloat16),
                        )
                        nc.gpsimd.collective_compute(
                            "AllGather", Alu.bypass,
                            replica_groups=[list(range(NCORES))],
                            ins=[cc_wi[:]], outs=[cc_wo[:]],
                        )
                    if g > 0:
                        # rs for the previous group's pairs: lagged one
                        # group so the PE queue never stalls on drains
                        for jp in range((g - 1) * G // 2, g * G // 2):
                            rs_pair(jp)
                    for a in range(G):
                        jc = g * G + a
                        pc = P_chunk(jc)
                        path = PAT[jc % len(PAT)]
                        if path == "D":
                            nc.vector.tensor_tensor(
                                out=pc, in0=st[:, a, 0, :], in1=st[:, a, 1, :],
                                op=Alu.subtract,
                            )
                            pcu = pc.bitcast(dt.uint16)
                            nc.vector.tensor_scalar(
                                out=pcu, in0=pcu, scalar1=0x7F7F,
                                scalar2=None, op0=Alu.bitwise_and,
                            )
                        else:
                            # one PSUM bank per half-chunk: finer drain
                            # granularity (drain h0 while h1's MM runs)
                            # and two fd=512 drains beat one fd=1024
                            for h in (0, 512):
                                ps = psp.tile(
                                    [128, 512], dt.float32, tag="ps"
                                )
                                nc.tensor.matmul(
                                    ps[:], idd_dr,
                                    st[:, a, :, h:h + 512],
                                    start=True, stop=True,
                                    perf_mode=mybir.MatmulPerfMode.DoubleRow,
                                )
                                if path == "A":
                                    nc.scalar.activation(
                                        out=pc[:, h:h + 512], in_=ps[:],
                                        func=Act.Abs,
                                    )
                                else:
                                    nc.vector.tensor_copy(
                                        out=pc[:, h:h + 512], in_=ps[:]
                                    )
                            if path == "V":
                                pcu = pc.bitcast(dt.uint16)
                                nc.vector.tensor_scalar(
                                    out=pcu, in0=pcu, scalar1=0x7F7F,
                                    scalar2=None, op0=Alu.bitwise_and,
                                )
                # final group's rs pairs
                for jp in range((NG - 1) * G // 2, NG * G // 2):
                    rs_pair(jp)

            # ---- row sums -> AllGather (natural order); the PSUM->SBUF
            # copy is single-lane, so split it across ACT+DVE halves to
            # halve the latency on the collective-trigger path ----
            nc.scalar.activation(
                out=rs_sb[:, 0:512], in_=rs_ps[:, 0:512], func=Act.Copy
            )
            nc.vector.tensor_copy(
                out=rs_sb[:, 512:NB], in_=rs_ps[:, 512:NB]
            )
            nc.sync.dma_start(
                out=cc_in[:].rearrange("(a f) -> a f", a=1), in_=rs_sb[:]
            )
            nc.gpsimd.collective_compute(
                "AllGather", Alu.bypass,
                replica_groups=[list(range(NCORES))],
                ins=[cc_in[:]], outs=[cc_out[:]],
            )

            r0v = nc.partition_id() * NB
            NP2 = NCH // 2  # 32 chunk pairs
            BP = (NB // CW) // 2  # 4 band pairs

            with contextlib.ExitStack() as p2:
                psp2 = p2.enter_context(
                    tc.tile_pool(name="psp2", bufs=1, space="PSUM")
                )
                tp_ps = psp2.tile([128, NCH], dt.bfloat16)
                tp8_ps = psp2.tile([128, 8], dt.bfloat16)
                y_ps = psp2.tile([1, NB], dt.float32)

                t_split = t_stat[:].rearrange("p (a jp) -> p a jp", a=2)

                def gemv(jp0, jp1):
                    for jp in range(jp0, jp1):
                        t_dr = t_split[:, :, jp:jp + 1]
                        for h in (0, 512):
                            nc.tensor.matmul(
                                y_ps[:, h:h + 512], t_dr, P_pair(jp, h),
                                start=(jp == 0), stop=(jp == NP2 - 1),
                                perf_mode=mybir.MatmulPerfMode.DoubleRow,
                            )

                # ---- band chunks (own columns): t needs only LOCAL rs,
                # so this entire sub-GEMV overlaps the AllGather ----
                rs_b8 = statp.tile([8, CW], dt.bfloat16)
                trc8 = statp.tile([8, CW], dt.float32)
                t8_bf = statp.tile([8, CW], dt.bfloat16)
                nc.scalar.dma_start(
                    out=rs_b8[:],
                    in_=cc_in[0:NB].rearrange("(c p) -> c p", c=8),
                )
                nc.vector.reciprocal(out=trc8[:], in_=rs_b8[:])
                nc.vector.tensor_tensor(
                    out=trc8[:], in0=trc8[:], in1=srot_sb[0:8, :], op=Alu.mult
                )
                nc.scalar.activation(
                    out=t8_bf[:], in_=trc8[:], func=Act.Copy, scale=TSCALE,
                )
                nc.tensor.transpose(tp8_ps[:], t8_bf[:], id64[0:8, 0:8])
                nc.vector.tensor_copy(
                    out=t_split[:, :, 0:BP],
                    in_=tp8_ps[:].rearrange("p (jp a) -> p a jp", a=2),
                )
                gemv(0, BP)

                # ---- t = s * recip(rs), rotated, to stationary layout ----
                nc.sync.dma_start(out=cc_dup[0:N], in_=cc_out[:])
                nc.scalar.dma_start(
                    out=cc_dup[N:N + 7 * NB], in_=cc_out[0:7 * NB]
                )
                # two half-window loads on two queues: recip of half 1
                # starts as soon as its 8 KB lands
                nc.sync.dma_start(
                    out=rs_rot[0:32, :],
                    in_=cc_dup[bass.ds(r0v, N // 2)].rearrange(
                        "(c p) -> c p", c=NCH // 2
                    ),
                )
                nc.scalar.dma_start(
                    out=rs_rot[32:64, :],
                    in_=cc_dup[bass.ds(r0v + N // 2, N // 2)].rearrange(
                        "(c p) -> c p", c=NCH // 2
                    ),
                )
                # two halves so the GEMV starts while the second half's
                # t is still being prepared
                t_hs = [
                    statp.tile([32, CW], dt.bfloat16, name=f"t_h{i}")
                    for i in range(2)
                ]
                for hi, (c0, c1) in enumerate(((0, 32), (32, 64))):
                    # j0: first non-band chunk of this half
                    j0 = max(c0, 8)
                    nc.vector.reciprocal(
                        out=trc[c0:c1, :], in_=rs_rot[c0:c1, :]
                    )
                    nc.vector.tensor_tensor(
                        out=trc[c0:c1, :], in0=trc[c0:c1, :],
                        in1=srot_sb[c0:c1, :], op=Alu.mult,
                    )
                    # scale t into fp8 range (undone in the final affine)
                    nc.scalar.activation(
                        out=t_hs[hi][:], in_=trc[c0:c1, :],
                        func=Act.Copy, scale=TSCALE,
                    )
                    # transpose [32, 128] -> [128, 32]
                    nc.tensor.transpose(
                        tp_ps[:, c0:c1], t_hs[hi][:], id64[0:32, 0:32],
                    )
                    # parity-split copy: t_stat[p, a*32+jp] = t[chunk 2jp+a]
                    # (DoubleRow lhsT planes must be >=16B apart); skip the
                    # band cols already placed above
                    nc.vector.tensor_copy(
                        out=t_split[:, :, j0 // 2:c1 // 2],
                        in_=tp_ps[:, c0:c1].rearrange(
                            "p (jp a) -> p a jp", a=2
                        )[:, :, (j0 - c0) // 2:],
                    )
                    # ---- phase 2: GEMV y[i] = sum_j P^T[j,i] t[j] ----
                    gemv(j0 // 2, c1 // 2)

                # out = (1-C)/TSCALE * y' + C/n, split ACT+DVE (single-
                # lane op; halves the final-affine latency)
                nc.scalar.activation(
                    out=out_sb[:, 0:512], in_=y_ps[:, 0:512], func=Act.Copy,
                    bias=float(C / N), scale=float((1.0 - C) / TSCALE),
                )
                nc.vector.tensor_scalar(
                    out=out_sb[:, 512:NB], in0=y_ps[:, 512:NB],
                    scalar1=float((1.0 - C) / TSCALE),
                    scalar2=float(C / N),
                    op0=Alu.mult, op1=Alu.add,
                )
            nc.sync.dma_start(
                out=OUT.ap().rearrange("(a f) -> a f", a=1), in_=out_sb[:]
            )

    nc.finalize()
    _built["nc"] = nc
    return nc


def _shard_inputs(B, sim, s):
    """Host packing: slice / transpose / select / pack in fp8, plus the
    constant tables the kernel loads (identities, ones)."""
    import ml_dtypes

    f8 = ml_dtypes.float8_e4m3
    bf16 = ml_dtypes.bfloat16
    B8 = B.astype(f8).view(np.uint8)
    sim8 = sim.astype(f8).view(np.uint8)
    s_ext = np.concatenate([s, s]).astype(np.float32)

    idd = np.zeros((128, 2 * CW), dtype=np.float32)
    idd[:, 0:CW] = np.eye(CW)
    idd[:, CW:2 * CW] = -np.eye(CW)
    idd = idd.astype(f8)
    id64 = np.eye(64, dtype=np.float32).astype(bf16)
    ones = np.ones((128, 32), dtype=np.float32).astype(f8)

    iidx = np.arange(NB)[None, :]

    in_maps = []
    for d in range(NCORES):
        r0, r1 = d * NB, (d + 1) * NB

        xs = np.empty((NG, 128, G, 2, NB), dtype=np.uint8)
        Bblk = B8[r0:r1, r0:r1]
        for jc in range(NCH):
            j0 = (r0 + jc * CW) % N
            if jc < NB // CW:
                # band chunk: straddle resolved by selection
                cs = slice(jc * CW, (jc + 1) * CW)
                m1t = np.ascontiguousarray(Bblk[:, cs].T)
                m2 = Bblk[cs, :]
                pidx = (jc * CW + np.arange(CW))[:, None]
                x = np.where(iidx < pidx, m1t, np.where(iidx > pidx, m2, 0))
            elif j0 >= r1:
                x = np.ascontiguousarray(B8[r0:r1, j0:j0 + CW].T)
            else:
                x = B8[j0:j0 + CW, r0:r1]
            st = np.ascontiguousarray(sim8[r0:r1, j0:j0 + CW].T)
            g, a = jc // G, jc % G
            xs[g, :, a, 0, :] = x
            xs[g, :, a, 1, :] = st

        srot = np.ascontiguousarray(s_ext[r0:r0 + N].reshape(NCH, CW))

        in_maps.append({
            "xs": xs.view(f8),
            "srot": srot,
            "idd": idd,
            "id64": id64,
            "ones": ones,
        })
    return in_maps


def kernel(B, similarity_matrix, connectivity_scores, _trace=False,
           _tmpdir=None):
    from concourse import bass_utils

    B = np.asarray(B, dtype=np.float32)
    sim = np.asarray(similarity_matrix, dtype=np.float32)
    s = np.asarray(connectivity_scores, dtype=np.float32)

    nc = _build()
    in_maps = _shard_inputs(B, sim, s)
    res = bass_utils.run_bass_kernel_spmd(
        nc, in_maps, core_ids=list(range(NCORES)), trace=_trace,
        tmpdir=_tmpdir
    )
    out = np.concatenate([res.results[d]["out"] for d in range(NCORES)])
    if _trace:
        kernel.last_results = res
    return out
